# revision 9
# baseline (speedup 1.0000x reference)
"""Trainium2 Bass kernel for nn_EnsembleTransitionModel.

Sharding: model-parallel (expert-parallel). M=8 ensemble members across 8
NeuronCores; each core runs one full MLP over the whole batch. Inputs are
replicated, per-model weights are sharded.

Device layout: activations are kept feature-major (x^T: [features, batch])
so every weight matrix is used directly as the matmul stationary operand
lhsT=[K_in, M_out] without any transposes. BatchNorm (eval) is folded into
a per-feature affine (scale, bias) applied by one scalar-engine Relu
activation straight out of PSUM. The residual z_last (= x rows 1536:1920)
rides a separate fp32 path so the dominant output term stays full precision.

Matmul dtype modes:
  bf16 — weights/activations bf16: LDWEIGHTS is a separate instruction the
         PE pulls ahead of in-flight matmuls, hiding the weight load.
  f32r — rounded fp32 (full-rate 1 cycle/row) but self-loading matmuls pay
         the ~85-cycle weight load serially.
"""

import os
import sys

import numpy as np

for _p in ("/opt/trn_rl_repo", "/root/.axon_site/_ro/trn_rl_repo"):
    if os.path.isdir(_p) and _p not in sys.path:
        sys.path.insert(0, _p)

M = 8
B = 16384
HIST = 5
L = 384
A = 1
HID = 512
NHL = 2
DIN = L * HIST + A * HIST  # 1925
EPS = 1e-5

DT_MODE = "bf16"  # "bf16" | "f32r"

NCH = 512  # batch columns per chunk (= max fp32 moving dim = 1 PSUM bank)
KT1 = 16  # DIN padded to 2048 = 16 full k-tiles
DINP = KT1 * 128
HT = HID // 128  # 4 hidden feature tiles
LT = L // 128  # 3 output feature tiles
ZROW0 = (HIST - 1) * L  # 1536: first row of z_last within x^T

# vecs columns: [b1 (4) | s0 (4) | c0 (4) | s1 (4) | c1 (4) | b3 (3)]
COL_B1 = 0
COL_S = lambda l: 4 + 8 * l
COL_C = lambda l: 8 + 8 * l
COL_B3 = 4 + 8 * NHL
NVEC = COL_B3 + LT


def build_bass(batch=B, dt_mode=DT_MODE):
    import concourse.bacc as bacc
    import concourse.tile as tile
    from concourse import mybir

    f32 = mybir.dt.float32
    mdt = mybir.dt.bfloat16 if dt_mode == "bf16" else mybir.dt.float32r
    Relu = mybir.ActivationFunctionType.Relu
    add = mybir.AluOpType.add

    nchunks = batch // NCH
    assert nchunks * NCH == batch

    nc = bacc.Bacc("TRN2", target_bir_lowering=False)
    xT = nc.declare_dram_parameter("xT", [DINP, batch], mdt, isOutput=False)
    zT = nc.declare_dram_parameter("zT", [L, batch], f32, isOutput=False)
    w1 = nc.declare_dram_parameter("w1", [128, KT1, HID], mdt, isOutput=False)
    wh = nc.declare_dram_parameter("wh", [128, NHL, HT, HT, 128], mdt, isOutput=False)
    w3 = nc.declare_dram_parameter("w3", [128, HT, L], mdt, isOutput=False)
    vecs = nc.declare_dram_parameter("vecs", [128, NVEC], f32, isOutput=False)
    outT = nc.declare_dram_parameter("outT", [L, batch], f32, isOutput=True)

    with tile.TileContext(nc) as tc:
        with (
            tc.tile_pool(name="wt", bufs=1) as wpool,
            tc.tile_pool(name="x", bufs=3) as xpool,
            tc.tile_pool(name="z", bufs=2) as zpool,
            tc.tile_pool(name="h", bufs=2) as hpool,
            tc.tile_pool(name="o", bufs=2) as opool,
            tc.tile_pool(name="ps", bufs=2, space="PSUM") as pspool,
            tc.tile_pool(name="psh", bufs=4, space="PSUM") as pshpool,
        ):
            # per-k-tile weight tiles so the first matmul only waits on its
            # own 256KB slice, not the whole 4MB preload
            w1_sb = []
            for kt in range(KT1):
                t = wpool.tile([128, HID], mdt, tag=f"w1_{kt}")
                nc.sync.dma_start(out=t[:], in_=w1[:, kt, :])
                w1_sb.append(t)
            # allocate now, DMA after chunk-0's x tiles are queued so the
            # first layer-1 matmuls aren't stuck behind these preloads
            wh_sb = wpool.tile([128, NHL, HT, HT, 128], mdt, tag="wh")
            w3_sb = wpool.tile([128, HT, L], mdt, tag="w3")
            v_sb = wpool.tile([128, NVEC], f32, tag="vecs")

            for c in range(nchunks):
                b0 = c * NCH
                # ---- stream x^T chunk: 16 k-tiles of [128, NCH] ----
                xts = []
                for kt in range(KT1):
                    xt = xpool.tile([128, NCH], mdt, tag=f"x{kt}")
                    nc.sync.dma_start(
                        out=xt[:],
                        in_=xT[kt * 128 : (kt + 1) * 128, b0 : b0 + NCH],
                    )
                    xts.append(xt)
                # fp32 z_last tiles for the residual path
                zts = []
                for lt in range(LT):
                    zt = zpool.tile([128, NCH], f32, tag=f"z{lt}")
                    nc.sync.dma_start(
                        out=zt[:],
                        in_=zT[lt * 128 : (lt + 1) * 128, b0 : b0 + NCH],
                    )
                    zts.append(zt)

                if c == 0:
                    nc.sync.dma_start(out=wh_sb[:], in_=wh[:])
                    nc.sync.dma_start(out=w3_sb[:], in_=w3[:])
                    nc.sync.dma_start(out=v_sb[:], in_=vecs[:])

                # ---- layer 1: h1 = relu(W1^T x + b1), [512, NCH] ----
                h1 = []
                for ht in range(HT):
                    ps = pspool.tile([128, NCH], f32, tag="ps1")
                    for kt in range(KT1):
                        nc.tensor.matmul(
                            ps[:],
                            w1_sb[kt][:, ht * 128 : (ht + 1) * 128],
                            xts[kt][:],
                            start=(kt == 0),
                            stop=(kt == KT1 - 1),
                        )
                    hsb = hpool.tile([128, NCH], mdt, tag=f"h1_{ht}")
                    nc.scalar.activation(
                        hsb[:], ps[:], Relu, bias=v_sb[:, COL_B1 + ht : COL_B1 + ht + 1]
                    )
                    h1.append(hsb)

                # ---- hidden layers: h = relu((h @ Wh[l]) * s_l + c_l) ----
                hin = h1
                for l in range(NHL):
                    hout = []
                    for mt in range(HT):
                        ps = pshpool.tile([128, NCH], f32, tag="ps2")
                        for kt in range(HT):
                            nc.tensor.matmul(
                                ps[:],
                                wh_sb[:, l, kt, mt, :],
                                hin[kt][:],
                                start=(kt == 0),
                                stop=(kt == HT - 1),
                            )
                        hsb = hpool.tile([128, NCH], mdt, tag=f"h{l + 2}_{mt}")
                        nc.scalar.activation(
                            hsb[:],
                            ps[:],
                            Relu,
                            bias=v_sb[:, COL_C(l) + mt : COL_C(l) + mt + 1],
                            scale=v_sb[:, COL_S(l) + mt : COL_S(l) + mt + 1],
                        )
                        hout.append(hsb)
                    hin = hout

                # ---- out: delta^T = W3^T h + b3; out = delta^T + zlast^T ----
                for lt in range(LT):
                    ps = pspool.tile([128, NCH], f32, tag="pso")
                    for kt in range(HT):
                        nc.tensor.matmul(
                            ps[:],
                            w3_sb[:, kt, lt * 128 : (lt + 1) * 128],
                            hin[kt][:],
                            start=(kt == 0),
                            stop=(kt == HT - 1),
                        )
                    ot = opool.tile([128, NCH], f32, tag=f"o{lt}")
                    nc.vector.tensor_tensor(ot[:], ps[:], zts[lt][:], add)
                    nc.vector.tensor_scalar_add(
                        ot[:], ot[:], v_sb[:, COL_B3 + lt : COL_B3 + lt + 1]
                    )
                    nc.sync.dma_start(
                        out=outT[lt * 128 : (lt + 1) * 128, b0 : b0 + NCH], in_=ot[:]
                    )
    nc.compile()
    return nc


def _mdt_np(dt_mode):
    if dt_mode == "bf16":
        import ml_dtypes

        return ml_dtypes.bfloat16
    return np.float32


def prep_core_inputs(
    z_hist, a_hist, W1, b1, Wh, bh, gamma, beta, rmean, rvar, W3, b3, dt_mode=DT_MODE
):
    """Host-side shard prep: returns per-model input dicts (xT/zT shared)."""
    mnp = _mdt_np(dt_mode)
    batch = z_hist.shape[0]
    x = np.concatenate(
        [z_hist.reshape(batch, -1), a_hist.reshape(batch, -1)], axis=1
    ).astype(np.float32)
    xT = np.zeros((DINP, batch), mnp)
    xT[:DIN] = x.T.astype(mnp)
    zT = np.ascontiguousarray(x.T[ZROW0 : ZROW0 + L]).astype(np.float32)

    rstd = 1.0 / np.sqrt(rvar.astype(np.float64) + EPS)  # [NHL, M, HID]
    s_aff = (gamma * rstd).astype(np.float32)
    c_aff = ((bh - rmean) * gamma * rstd + beta).astype(np.float32)

    in_maps = []
    for m in range(M):
        w1p = np.zeros((DINP, HID), np.float32)
        w1p[:DIN] = W1[m]
        w1h = np.ascontiguousarray(
            w1p.reshape(KT1, 128, HID).transpose(1, 0, 2)
        ).astype(mnp)  # [128, KT1, HID]

        whh = np.ascontiguousarray(
            Wh[:, m].reshape(NHL, HT, 128, HT, 128).transpose(2, 0, 1, 3, 4)
        ).astype(mnp)  # [128, NHL, kt, mt, 128]

        w3h = np.ascontiguousarray(W3[m].reshape(HT, 128, L).transpose(1, 0, 2)).astype(
            mnp
        )  # [128, HT, L]

        vecs = np.zeros((128, NVEC), np.float32)
        vecs[:, COL_B1 : COL_B1 + HT] = b1[m].reshape(HT, 128).T
        for l in range(NHL):
            vecs[:, COL_S(l) : COL_S(l) + HT] = s_aff[l, m].reshape(HT, 128).T
            vecs[:, COL_C(l) : COL_C(l) + HT] = c_aff[l, m].reshape(HT, 128).T
        vecs[:, COL_B3 : COL_B3 + LT] = b3[m].reshape(LT, 128).T

        in_maps.append(
            {"xT": xT, "zT": zT, "w1": w1h, "wh": whh, "w3": w3h, "vecs": vecs}
        )
    return in_maps


def kernel(**inputs):
    inputs = {k: np.asarray(v) for k, v in inputs.items()}
    in_maps = prep_core_inputs(**inputs)
    nc = build_bass(B)

    from concourse import bass_utils

    res = bass_utils.run_bass_kernel_spmd(nc, in_maps, core_ids=list(range(M)))
    out = np.stack(
        [np.ascontiguousarray(res.results[m]["outT"].T) for m in range(M)]
    )  # [M, B, L]
    return out.astype(np.float32)


# revision 10
# speedup vs baseline: 1.0411x; 1.0411x over previous
"""Trainium2 Bass kernel for nn_EnsembleTransitionModel.

Sharding: model-parallel (expert-parallel). M=8 ensemble members across 8
NeuronCores; each core runs one full MLP over the whole batch. Inputs are
replicated, per-model weights are sharded.

Device layout: activations are kept feature-major (x^T: [features, batch])
so every weight matrix is used directly as the matmul stationary operand
lhsT=[K_in, M_out] without any transposes. BatchNorm (eval) is folded into
a per-feature affine (scale, bias) applied by one scalar-engine Relu
activation straight out of PSUM. The residual z_last (= x rows 1536:1920)
rides a separate fp32 path so the dominant output term stays full precision.

Matmul dtype modes:
  bf16 — weights/activations bf16: LDWEIGHTS is a separate instruction the
         PE pulls ahead of in-flight matmuls, hiding the weight load.
  f32r — rounded fp32 (full-rate 1 cycle/row) but self-loading matmuls pay
         the ~85-cycle weight load serially.
"""

import os
import sys

import numpy as np

for _p in ("/opt/trn_rl_repo", "/root/.axon_site/_ro/trn_rl_repo"):
    if os.path.isdir(_p) and _p not in sys.path:
        sys.path.insert(0, _p)

M = 8
B = 16384
HIST = 5
L = 384
A = 1
HID = 512
NHL = 2
DIN = L * HIST + A * HIST  # 1925
EPS = 1e-5

DT_MODE = "bf16"  # "bf16" | "f32r"

NCH = 512  # batch columns per chunk (= max fp32 moving dim = 1 PSUM bank)
KT1 = 16  # x^T padded to 2048 rows; device uses k-tiles 0..14 (z part) only:
KTZ = 15  # the 5 a_hist rows (a rank-5 term) are precomputed on host as ya
DINP = KT1 * 128
HT = HID // 128  # 4 hidden feature tiles
LT = L // 128  # 3 output feature tiles
ZROW0 = (HIST - 1) * L  # 1536: first row of z_last within x^T

# vecs columns: [b1 (4) | s0 (4) | c0 (4) | s1 (4) | c1 (4) | b3 (3)]
COL_B1 = 0
COL_S = lambda l: 4 + 8 * l
COL_C = lambda l: 8 + 8 * l
COL_B3 = 4 + 8 * NHL
NVEC = COL_B3 + LT


def build_bass(batch=B, dt_mode=DT_MODE):
    import concourse.bacc as bacc
    import concourse.tile as tile
    from concourse import mybir

    f32 = mybir.dt.float32
    mdt = mybir.dt.bfloat16 if dt_mode == "bf16" else mybir.dt.float32r
    Relu = mybir.ActivationFunctionType.Relu
    add = mybir.AluOpType.add

    nchunks = batch // NCH
    assert nchunks * NCH == batch

    nc = bacc.Bacc("TRN2", target_bir_lowering=False)
    xT = nc.declare_dram_parameter("xT", [DINP, batch], mdt, isOutput=False)
    zT = nc.declare_dram_parameter("zT", [L, batch], f32, isOutput=False)
    ya = nc.declare_dram_parameter("ya", [128, HT, batch], mdt, isOutput=False)
    w1 = nc.declare_dram_parameter("w1", [128, KT1, HID], mdt, isOutput=False)
    wh = nc.declare_dram_parameter("wh", [128, NHL, HT, HT, 128], mdt, isOutput=False)
    w3 = nc.declare_dram_parameter("w3", [128, HT, L], mdt, isOutput=False)
    vecs = nc.declare_dram_parameter("vecs", [128, NVEC], f32, isOutput=False)
    outT = nc.declare_dram_parameter("outT", [L, batch], f32, isOutput=True)

    with tile.TileContext(nc) as tc:
        with (
            tc.tile_pool(name="wt", bufs=1) as wpool,
            tc.tile_pool(name="x", bufs=2) as xpool,
            tc.tile_pool(name="z", bufs=2) as zpool,
            tc.tile_pool(name="h", bufs=2) as hpool,
            tc.tile_pool(name="o", bufs=2) as opool,
            tc.tile_pool(name="ps", bufs=2, space="PSUM") as pspool,
            tc.tile_pool(name="psh", bufs=4, space="PSUM") as pshpool,
        ):
            # per-k-tile weight tiles so the first matmul only waits on its
            # own 256KB slice, not the whole 4MB preload
            w1_sb = []
            for kt in range(KTZ):
                t = wpool.tile([128, HID], mdt, tag=f"w1_{kt}")
                nc.sync.dma_start(out=t[:], in_=w1[:, kt, :])
                w1_sb.append(t)
            # allocate now, DMA after chunk-0's x tiles are queued so the
            # first layer-1 matmuls aren't stuck behind these preloads
            wh_sb = wpool.tile([128, NHL, HT, HT, 128], mdt, tag="wh")
            w3_sb = wpool.tile([128, HT, L], mdt, tag="w3")
            v_sb = wpool.tile([128, NVEC], f32, tag="vecs")

            for c in range(nchunks):
                b0 = c * NCH
                # ---- stream x^T chunk: 16 k-tiles of [128, NCH] ----
                xts = []
                for kt in range(KTZ):
                    xt = xpool.tile([128, NCH], mdt, tag=f"x{kt}")
                    nc.sync.dma_start(
                        out=xt[:],
                        in_=xT[kt * 128 : (kt + 1) * 128, b0 : b0 + NCH],
                    )
                    xts.append(xt)
                yas = []
                for ht in range(HT):
                    yt = zpool.tile([128, NCH], mdt, tag=f"ya{ht}")
                    nc.sync.dma_start(
                        out=yt[:], in_=ya[:, ht, b0 : b0 + NCH]
                    )
                    yas.append(yt)
                # fp32 z_last tiles for the residual path
                zts = []
                for lt in range(LT):
                    zt = zpool.tile([128, NCH], f32, tag=f"z{lt}")
                    nc.sync.dma_start(
                        out=zt[:],
                        in_=zT[lt * 128 : (lt + 1) * 128, b0 : b0 + NCH],
                    )
                    zts.append(zt)

                if c == 0:
                    nc.sync.dma_start(out=wh_sb[:], in_=wh[:])
                    nc.sync.dma_start(out=w3_sb[:], in_=w3[:])
                    nc.sync.dma_start(out=v_sb[:], in_=vecs[:])

                # ---- layer 1: h1 = relu(W1^T x + b1), [512, NCH] ----
                h1 = []
                for ht in range(HT):
                    ps = pspool.tile([128, NCH], f32, tag="ps1")
                    for kt in range(KTZ):
                        nc.tensor.matmul(
                            ps[:],
                            w1_sb[kt][:, ht * 128 : (ht + 1) * 128],
                            xts[kt][:],
                            start=(kt == 0),
                            stop=(kt == KTZ - 1),
                        )
                    tsb = hpool.tile([128, NCH], f32, tag=f"t1_{ht}")
                    nc.vector.tensor_tensor(tsb[:], ps[:], yas[ht][:], add)
                    hsb = hpool.tile([128, NCH], mdt, tag=f"h1_{ht}")
                    nc.scalar.activation(
                        hsb[:], tsb[:], Relu, bias=v_sb[:, COL_B1 + ht : COL_B1 + ht + 1]
                    )
                    h1.append(hsb)

                # ---- hidden layers: h = relu((h @ Wh[l]) * s_l + c_l) ----
                hin = h1
                for l in range(NHL):
                    hout = []
                    for mt in range(HT):
                        ps = pshpool.tile([128, NCH], f32, tag="ps2")
                        for kt in range(HT):
                            nc.tensor.matmul(
                                ps[:],
                                wh_sb[:, l, kt, mt, :],
                                hin[kt][:],
                                start=(kt == 0),
                                stop=(kt == HT - 1),
                            )
                        hsb = hpool.tile([128, NCH], mdt, tag=f"h{l + 2}_{mt}")
                        nc.scalar.activation(
                            hsb[:],
                            ps[:],
                            Relu,
                            bias=v_sb[:, COL_C(l) + mt : COL_C(l) + mt + 1],
                            scale=v_sb[:, COL_S(l) + mt : COL_S(l) + mt + 1],
                        )
                        hout.append(hsb)
                    hin = hout

                # ---- out: delta^T = W3^T h + b3; out = delta^T + zlast^T ----
                for lt in range(LT):
                    ps = pspool.tile([128, NCH], f32, tag="pso")
                    for kt in range(HT):
                        nc.tensor.matmul(
                            ps[:],
                            w3_sb[:, kt, lt * 128 : (lt + 1) * 128],
                            hin[kt][:],
                            start=(kt == 0),
                            stop=(kt == HT - 1),
                        )
                    ot = opool.tile([128, NCH], f32, tag=f"o{lt}")
                    nc.vector.tensor_tensor(ot[:], ps[:], zts[lt][:], add)
                    nc.vector.tensor_scalar_add(
                        ot[:], ot[:], v_sb[:, COL_B3 + lt : COL_B3 + lt + 1]
                    )
                    nc.sync.dma_start(
                        out=outT[lt * 128 : (lt + 1) * 128, b0 : b0 + NCH], in_=ot[:]
                    )
    nc.compile()
    return nc


def _mdt_np(dt_mode):
    if dt_mode == "bf16":
        import ml_dtypes

        return ml_dtypes.bfloat16
    return np.float32


def prep_core_inputs(
    z_hist, a_hist, W1, b1, Wh, bh, gamma, beta, rmean, rvar, W3, b3, dt_mode=DT_MODE
):
    """Host-side shard prep: returns per-model input dicts (xT/zT shared)."""
    mnp = _mdt_np(dt_mode)
    batch = z_hist.shape[0]
    x = np.concatenate(
        [z_hist.reshape(batch, -1), a_hist.reshape(batch, -1)], axis=1
    ).astype(np.float32)
    xT = np.zeros((DINP, batch), mnp)
    xT[:DIN] = x.T.astype(mnp)
    a_flat = x[:, KTZ * 128 :]  # [batch, 5]
    zT = np.ascontiguousarray(x.T[ZROW0 : ZROW0 + L]).astype(np.float32)

    rstd = 1.0 / np.sqrt(rvar.astype(np.float64) + EPS)  # [NHL, M, HID]
    s_aff = (gamma * rstd).astype(np.float32)
    c_aff = ((bh - rmean) * gamma * rstd + beta).astype(np.float32)

    in_maps = []
    for m in range(M):
        w1p = np.zeros((DINP, HID), np.float32)
        w1p[:DIN] = W1[m]
        w1h = np.ascontiguousarray(
            w1p.reshape(KT1, 128, HID).transpose(1, 0, 2)
        ).astype(mnp)  # [128, KT1, HID]

        whh = np.ascontiguousarray(
            Wh[:, m].reshape(NHL, HT, 128, HT, 128).transpose(2, 0, 1, 3, 4)
        ).astype(mnp)  # [128, NHL, kt, mt, 128]

        w3h = np.ascontiguousarray(W3[m].reshape(HT, 128, L).transpose(1, 0, 2)).astype(
            mnp
        )  # [128, HT, L]

        vecs = np.zeros((128, NVEC), np.float32)
        vecs[:, COL_B1 : COL_B1 + HT] = b1[m].reshape(HT, 128).T
        for l in range(NHL):
            vecs[:, COL_S(l) : COL_S(l) + HT] = s_aff[l, m].reshape(HT, 128).T
            vecs[:, COL_C(l) : COL_C(l) + HT] = c_aff[l, m].reshape(HT, 128).T
        vecs[:, COL_B3 : COL_B3 + LT] = b3[m].reshape(LT, 128).T

        y_a = (a_flat @ W1[m][KTZ * 128 :]).T  # [HID, batch] fp32
        yah = np.ascontiguousarray(
            y_a.reshape(HT, 128, batch).transpose(1, 0, 2)
        ).astype(mnp)  # [128, HT, batch]
        in_maps.append(
            {"xT": xT, "zT": zT, "ya": yah, "w1": w1h, "wh": whh, "w3": w3h,
             "vecs": vecs}
        )
    return in_maps


def kernel(**inputs):
    inputs = {k: np.asarray(v) for k, v in inputs.items()}
    in_maps = prep_core_inputs(**inputs)
    nc = build_bass(B)

    from concourse import bass_utils

    res = bass_utils.run_bass_kernel_spmd(nc, in_maps, core_ids=list(range(M)))
    out = np.stack(
        [np.ascontiguousarray(res.results[m]["outT"].T) for m in range(M)]
    )  # [M, B, L]
    return out.astype(np.float32)


# revision 11
# speedup vs baseline: 1.0445x; 1.0032x over previous
"""Trainium2 Bass kernel for nn_EnsembleTransitionModel.

Sharding: model-parallel (expert-parallel). M=8 ensemble members across 8
NeuronCores; each core runs one full MLP over the whole batch. Inputs are
replicated, per-model weights are sharded.

Device layout: activations are kept feature-major (x^T: [features, batch])
so every weight matrix is used directly as the matmul stationary operand
lhsT=[K_in, M_out] without any transposes. BatchNorm (eval) is folded into
a per-feature affine (scale, bias) applied by one scalar-engine Relu
activation straight out of PSUM. The residual z_last (= x rows 1536:1920)
rides a separate fp32 path so the dominant output term stays full precision.

Matmul dtype modes:
  bf16 — weights/activations bf16: LDWEIGHTS is a separate instruction the
         PE pulls ahead of in-flight matmuls, hiding the weight load.
  f32r — rounded fp32 (full-rate 1 cycle/row) but self-loading matmuls pay
         the ~85-cycle weight load serially.
"""

import os
import sys

import numpy as np

for _p in ("/opt/trn_rl_repo", "/root/.axon_site/_ro/trn_rl_repo"):
    if os.path.isdir(_p) and _p not in sys.path:
        sys.path.insert(0, _p)

M = 8
B = 16384
HIST = 5
L = 384
A = 1
HID = 512
NHL = 2
DIN = L * HIST + A * HIST  # 1925
EPS = 1e-5

DT_MODE = "bf16"  # "bf16" | "f32r"

NCH = 512  # batch columns per chunk (= max fp32 moving dim = 1 PSUM bank)
KT1 = 16  # x^T padded to 2048 rows; device uses k-tiles 0..14 (z part) only:
KTZ = 15  # the 5 a_hist rows (a rank-5 term) are precomputed on host as ya
DINP = KT1 * 128
HT = HID // 128  # 4 hidden feature tiles
LT = L // 128  # 3 output feature tiles
ZROW0 = (HIST - 1) * L  # 1536: first row of z_last within x^T

# vecs columns: [b1 (4) | s0 (4) | c0 (4) | s1 (4) | c1 (4) | b3 (3)]
COL_B1 = 0
COL_S = lambda l: 4 + 8 * l
COL_C = lambda l: 8 + 8 * l
COL_B3 = 4 + 8 * NHL
NVEC = COL_B3 + LT


def build_bass(batch=B, dt_mode=DT_MODE):
    import concourse.bacc as bacc
    import concourse.tile as tile
    from concourse import mybir

    f32 = mybir.dt.float32
    mdt = mybir.dt.bfloat16 if dt_mode == "bf16" else mybir.dt.float32r
    Relu = mybir.ActivationFunctionType.Relu
    add = mybir.AluOpType.add

    nchunks = batch // NCH
    assert nchunks * NCH == batch

    nc = bacc.Bacc("TRN2", target_bir_lowering=False)
    xT = nc.declare_dram_parameter("xT", [DINP, batch], mdt, isOutput=False)
    zT = nc.declare_dram_parameter("zT", [L, batch], f32, isOutput=False)
    ya = nc.declare_dram_parameter("ya", [128, HT, batch], mdt, isOutput=False)
    w1 = nc.declare_dram_parameter("w1", [128, KT1, HID], mdt, isOutput=False)
    wh = nc.declare_dram_parameter("wh", [128, NHL, HT, HT, 128], mdt, isOutput=False)
    w3 = nc.declare_dram_parameter("w3", [128, HT, L], mdt, isOutput=False)
    vecs = nc.declare_dram_parameter("vecs", [128, NVEC], f32, isOutput=False)
    outT = nc.declare_dram_parameter("outT", [L, batch], f32, isOutput=True)

    with tile.TileContext(nc) as tc:
        with (
            tc.tile_pool(name="wt", bufs=1) as wpool,
            tc.tile_pool(name="x", bufs=2) as xpool,
            tc.tile_pool(name="z", bufs=2) as zpool,
            tc.tile_pool(name="h", bufs=2) as hpool,
            tc.tile_pool(name="o", bufs=2) as opool,
            tc.tile_pool(name="ps", bufs=2, space="PSUM") as pspool,
            tc.tile_pool(name="ps1", bufs=3, space="PSUM") as ps1pool,
            tc.tile_pool(name="psh", bufs=3, space="PSUM") as pshpool,
        ):
            # per-k-tile weight tiles so the first matmul only waits on its
            # own 256KB slice, not the whole 4MB preload
            w1_sb = []
            for kt in range(KTZ):
                t = wpool.tile([128, HID], mdt, tag=f"w1_{kt}")
                nc.sync.dma_start(out=t[:], in_=w1[:, kt, :])
                w1_sb.append(t)
            # allocate now, DMA after chunk-0's x tiles are queued so the
            # first layer-1 matmuls aren't stuck behind these preloads
            wh_sb = wpool.tile([128, NHL, HT, HT, 128], mdt, tag="wh")
            w3_sb = wpool.tile([128, HT, L], mdt, tag="w3")
            v_sb = wpool.tile([128, NVEC], f32, tag="vecs")

            for c in range(nchunks):
                b0 = c * NCH
                # small per-chunk streams first: the L1 psum recycle waits on
                # the ya add, and the out stage on z — don't queue them last
                ya_t = zpool.tile([128, HT, NCH], mdt, tag="ya")
                nc.sync.dma_start(out=ya_t[:], in_=ya[:, :, b0 : b0 + NCH])
                yas = [ya_t[:, ht, :] for ht in range(HT)]
                zts = []
                for lt in range(LT):
                    zt = zpool.tile([128, NCH], f32, tag=f"z{lt}")
                    nc.sync.dma_start(
                        out=zt[:],
                        in_=zT[lt * 128 : (lt + 1) * 128, b0 : b0 + NCH],
                    )
                    zts.append(zt)
                # ---- stream x^T chunk: 15 k-tiles of [128, NCH] ----
                xts = []
                for kt in range(KTZ):
                    xt = xpool.tile([128, NCH], mdt, tag=f"x{kt}")
                    nc.sync.dma_start(
                        out=xt[:],
                        in_=xT[kt * 128 : (kt + 1) * 128, b0 : b0 + NCH],
                    )
                    xts.append(xt)

                if c == 0:
                    nc.sync.dma_start(out=wh_sb[:], in_=wh[:])
                    nc.sync.dma_start(out=w3_sb[:], in_=w3[:])
                    nc.sync.dma_start(out=v_sb[:], in_=vecs[:])

                # ---- layer 1: h1 = relu(W1^T x + b1), [512, NCH] ----
                h1 = []
                for ht in range(HT):
                    ps = ps1pool.tile([128, NCH], f32, tag="ps1")
                    for kt in range(KTZ):
                        nc.tensor.matmul(
                            ps[:],
                            w1_sb[kt][:, ht * 128 : (ht + 1) * 128],
                            xts[kt][:],
                            start=(kt == 0),
                            stop=(kt == KTZ - 1),
                        )
                    tsb = hpool.tile([128, NCH], f32, tag=f"t1_{ht}")
                    nc.vector.tensor_tensor(tsb[:], ps[:], yas[ht], add)
                    hsb = hpool.tile([128, NCH], mdt, tag=f"h1_{ht}")
                    nc.scalar.activation(
                        hsb[:], tsb[:], Relu, bias=v_sb[:, COL_B1 + ht : COL_B1 + ht + 1]
                    )
                    h1.append(hsb)

                # ---- hidden layers: h = relu((h @ Wh[l]) * s_l + c_l) ----
                hin = h1
                for l in range(NHL):
                    hout = []
                    for mt in range(HT):
                        ps = pshpool.tile([128, NCH], f32, tag="ps2")
                        for kt in range(HT):
                            nc.tensor.matmul(
                                ps[:],
                                wh_sb[:, l, kt, mt, :],
                                hin[kt][:],
                                start=(kt == 0),
                                stop=(kt == HT - 1),
                            )
                        hsb = hpool.tile([128, NCH], mdt, tag=f"h{l + 2}_{mt}")
                        nc.scalar.activation(
                            hsb[:],
                            ps[:],
                            Relu,
                            bias=v_sb[:, COL_C(l) + mt : COL_C(l) + mt + 1],
                            scale=v_sb[:, COL_S(l) + mt : COL_S(l) + mt + 1],
                        )
                        hout.append(hsb)
                    hin = hout

                # ---- out: delta^T = W3^T h + b3; out = delta^T + zlast^T ----
                for lt in range(LT):
                    ps = pspool.tile([128, NCH], f32, tag="pso")
                    for kt in range(HT):
                        nc.tensor.matmul(
                            ps[:],
                            w3_sb[:, kt, lt * 128 : (lt + 1) * 128],
                            hin[kt][:],
                            start=(kt == 0),
                            stop=(kt == HT - 1),
                        )
                    ot = opool.tile([128, NCH], f32, tag=f"o{lt}")
                    nc.vector.tensor_tensor(ot[:], ps[:], zts[lt][:], add)
                    nc.vector.tensor_scalar_add(
                        ot[:], ot[:], v_sb[:, COL_B3 + lt : COL_B3 + lt + 1]
                    )
                    nc.sync.dma_start(
                        out=outT[lt * 128 : (lt + 1) * 128, b0 : b0 + NCH], in_=ot[:]
                    )
    nc.compile()
    return nc


def _mdt_np(dt_mode):
    if dt_mode == "bf16":
        import ml_dtypes

        return ml_dtypes.bfloat16
    return np.float32


def prep_core_inputs(
    z_hist, a_hist, W1, b1, Wh, bh, gamma, beta, rmean, rvar, W3, b3, dt_mode=DT_MODE
):
    """Host-side shard prep: returns per-model input dicts (xT/zT shared)."""
    mnp = _mdt_np(dt_mode)
    batch = z_hist.shape[0]
    x = np.concatenate(
        [z_hist.reshape(batch, -1), a_hist.reshape(batch, -1)], axis=1
    ).astype(np.float32)
    xT = np.zeros((DINP, batch), mnp)
    xT[:DIN] = x.T.astype(mnp)
    a_flat = x[:, KTZ * 128 :]  # [batch, 5]
    zT = np.ascontiguousarray(x.T[ZROW0 : ZROW0 + L]).astype(np.float32)

    rstd = 1.0 / np.sqrt(rvar.astype(np.float64) + EPS)  # [NHL, M, HID]
    s_aff = (gamma * rstd).astype(np.float32)
    c_aff = ((bh - rmean) * gamma * rstd + beta).astype(np.float32)

    in_maps = []
    for m in range(M):
        w1p = np.zeros((DINP, HID), np.float32)
        w1p[:DIN] = W1[m]
        w1h = np.ascontiguousarray(
            w1p.reshape(KT1, 128, HID).transpose(1, 0, 2)
        ).astype(mnp)  # [128, KT1, HID]

        whh = np.ascontiguousarray(
            Wh[:, m].reshape(NHL, HT, 128, HT, 128).transpose(2, 0, 1, 3, 4)
        ).astype(mnp)  # [128, NHL, kt, mt, 128]

        w3h = np.ascontiguousarray(W3[m].reshape(HT, 128, L).transpose(1, 0, 2)).astype(
            mnp
        )  # [128, HT, L]

        vecs = np.zeros((128, NVEC), np.float32)
        vecs[:, COL_B1 : COL_B1 + HT] = b1[m].reshape(HT, 128).T
        for l in range(NHL):
            vecs[:, COL_S(l) : COL_S(l) + HT] = s_aff[l, m].reshape(HT, 128).T
            vecs[:, COL_C(l) : COL_C(l) + HT] = c_aff[l, m].reshape(HT, 128).T
        vecs[:, COL_B3 : COL_B3 + LT] = b3[m].reshape(LT, 128).T

        y_a = (a_flat @ W1[m][KTZ * 128 :]).T  # [HID, batch] fp32
        yah = np.ascontiguousarray(
            y_a.reshape(HT, 128, batch).transpose(1, 0, 2)
        ).astype(mnp)  # [128, HT, batch]
        in_maps.append(
            {"xT": xT, "zT": zT, "ya": yah, "w1": w1h, "wh": whh, "w3": w3h,
             "vecs": vecs}
        )
    return in_maps


def kernel(**inputs):
    inputs = {k: np.asarray(v) for k, v in inputs.items()}
    in_maps = prep_core_inputs(**inputs)
    nc = build_bass(B)

    from concourse import bass_utils

    res = bass_utils.run_bass_kernel_spmd(nc, in_maps, core_ids=list(range(M)))
    out = np.stack(
        [np.ascontiguousarray(res.results[m]["outT"].T) for m in range(M)]
    )  # [M, B, L]
    return out.astype(np.float32)


# revision 13
# speedup vs baseline: 1.0451x; 1.0006x over previous
"""Trainium2 Bass kernel for nn_EnsembleTransitionModel.

Sharding: model-parallel (expert-parallel). M=8 ensemble members across 8
NeuronCores; each core runs one full MLP over the whole batch. Inputs are
replicated, per-model weights are sharded.

Device layout: activations are kept feature-major (x^T: [features, batch])
so every weight matrix is used directly as the matmul stationary operand
lhsT=[K_in, M_out] without any transposes. BatchNorm (eval) is folded into
a per-feature affine (scale, bias) applied by one scalar-engine Relu
activation straight out of PSUM. The residual z_last (= x rows 1536:1920)
rides a separate fp32 path so the dominant output term stays full precision.

Matmul dtype modes:
  bf16 — weights/activations bf16: LDWEIGHTS is a separate instruction the
         PE pulls ahead of in-flight matmuls, hiding the weight load.
  f32r — rounded fp32 (full-rate 1 cycle/row) but self-loading matmuls pay
         the ~85-cycle weight load serially.
"""

import os
import sys

import numpy as np

for _p in ("/opt/trn_rl_repo", "/root/.axon_site/_ro/trn_rl_repo"):
    if os.path.isdir(_p) and _p not in sys.path:
        sys.path.insert(0, _p)

M = 8
B = 16384
HIST = 5
L = 384
A = 1
HID = 512
NHL = 2
DIN = L * HIST + A * HIST  # 1925
EPS = 1e-5

DT_MODE = "bf16"  # "bf16" | "f32r"

NCH = 512  # batch columns per chunk (= max fp32 moving dim = 1 PSUM bank)
KT1 = 16  # x^T padded to 2048 rows; device uses k-tiles 0..14 (z part) only:
KTZ = 15  # the 5 a_hist rows (a rank-5 term) are precomputed on host as ya
DINP = KT1 * 128
HT = HID // 128  # 4 hidden feature tiles
LT = L // 128  # 3 output feature tiles
ZROW0 = (HIST - 1) * L  # 1536: first row of z_last within x^T

# vecs columns: [b1 (4) | s0 (4) | c0 (4) | s1 (4) | c1 (4) | b3 (3)]
COL_B1 = 0
COL_S = lambda l: 4 + 8 * l
COL_C = lambda l: 8 + 8 * l
COL_B3 = 4 + 8 * NHL
NVEC = COL_B3 + LT


def build_bass(batch=B, dt_mode=DT_MODE):
    import concourse.bacc as bacc
    import concourse.tile as tile
    from concourse import mybir

    f32 = mybir.dt.float32
    mdt = mybir.dt.bfloat16 if dt_mode == "bf16" else mybir.dt.float32r
    Relu = mybir.ActivationFunctionType.Relu
    add = mybir.AluOpType.add

    nchunks = batch // NCH
    assert nchunks * NCH == batch

    nc = bacc.Bacc("TRN2", target_bir_lowering=False)
    xT = nc.declare_dram_parameter("xT", [DINP, batch], mdt, isOutput=False)
    zT = nc.declare_dram_parameter("zT", [L, batch], f32, isOutput=False)
    ya = nc.declare_dram_parameter("ya", [128, HT, batch], mdt, isOutput=False)
    w1 = nc.declare_dram_parameter("w1", [128, KT1, HID], mdt, isOutput=False)
    wh = nc.declare_dram_parameter("wh", [128, NHL, HT, HT, 128], mdt, isOutput=False)
    w3 = nc.declare_dram_parameter("w3", [128, HT, L], mdt, isOutput=False)
    vecs = nc.declare_dram_parameter("vecs", [128, NVEC], f32, isOutput=False)
    outT = nc.declare_dram_parameter("outT", [L, batch], f32, isOutput=True)

    with tile.TileContext(nc) as tc:
        with (
            tc.tile_pool(name="wt", bufs=1) as wpool,
            tc.tile_pool(name="x", bufs=2) as xpool,
            tc.tile_pool(name="z", bufs=2) as zpool,
            tc.tile_pool(name="h", bufs=3) as hpool,
            tc.tile_pool(name="o", bufs=3) as opool,
            tc.tile_pool(name="ps", bufs=2, space="PSUM") as pspool,
            tc.tile_pool(name="ps1", bufs=3, space="PSUM") as ps1pool,
            tc.tile_pool(name="psh", bufs=3, space="PSUM") as pshpool,
        ):
            # per-k-tile weight tiles so the first matmul only waits on its
            # own 256KB slice, not the whole 4MB preload
            w1_sb = []
            for kt in range(KTZ):
                t = wpool.tile([128, HID], mdt, tag=f"w1_{kt}")
                nc.sync.dma_start(out=t[:], in_=w1[:, kt, :])
                w1_sb.append(t)
            # allocate now, DMA after chunk-0's x tiles are queued so the
            # first layer-1 matmuls aren't stuck behind these preloads
            wh_sb = wpool.tile([128, NHL, HT, HT, 128], mdt, tag="wh")
            w3_sb = wpool.tile([128, HT, L], mdt, tag="w3")
            v_sb = wpool.tile([128, NVEC], f32, tag="vecs")

            for c in range(nchunks):
                b0 = c * NCH
                # small per-chunk streams first: the L1 psum recycle waits on
                # the ya add, and the out stage on z — don't queue them last
                ya_t = zpool.tile([128, HT, NCH], mdt, tag="ya")
                nc.sync.dma_start(out=ya_t[:], in_=ya[:, :, b0 : b0 + NCH])
                yas = [ya_t[:, ht, :] for ht in range(HT)]
                zts = []
                for lt in range(LT):
                    zt = zpool.tile([128, NCH], f32, tag=f"z{lt}")
                    nc.sync.dma_start(
                        out=zt[:],
                        in_=zT[lt * 128 : (lt + 1) * 128, b0 : b0 + NCH],
                    )
                    zts.append(zt)
                # ---- stream x^T chunk: 15 k-tiles of [128, NCH] ----
                xts = []
                for kt in range(KTZ):
                    xt = xpool.tile([128, NCH], mdt, tag=f"x{kt}")
                    nc.sync.dma_start(
                        out=xt[:],
                        in_=xT[kt * 128 : (kt + 1) * 128, b0 : b0 + NCH],
                    )
                    xts.append(xt)

                if c == 0:
                    nc.sync.dma_start(out=wh_sb[:], in_=wh[:])
                    nc.sync.dma_start(out=w3_sb[:], in_=w3[:])
                    nc.sync.dma_start(out=v_sb[:], in_=vecs[:])

                # ---- layer 1: h1 = relu(W1^T x + b1), [512, NCH] ----
                h1 = []
                for ht in range(HT):
                    ps = ps1pool.tile([128, NCH], f32, tag="ps1")
                    for kt in range(KTZ):
                        nc.tensor.matmul(
                            ps[:],
                            w1_sb[kt][:, ht * 128 : (ht + 1) * 128],
                            xts[kt][:],
                            start=(kt == 0),
                            stop=(kt == KTZ - 1),
                        )
                    tsb = hpool.tile([128, NCH], f32, tag=f"t1_{ht}")
                    nc.vector.tensor_tensor(tsb[:], ps[:], yas[ht], add)
                    hsb = hpool.tile([128, NCH], mdt, tag=f"h1_{ht}")
                    nc.scalar.activation(
                        hsb[:], tsb[:], Relu, bias=v_sb[:, COL_B1 + ht : COL_B1 + ht + 1]
                    )
                    h1.append(hsb)

                # ---- hidden layers: h = relu((h @ Wh[l]) * s_l + c_l) ----
                hin = h1
                for l in range(NHL):
                    hout = []
                    for mt in range(HT):
                        ps = pshpool.tile([128, NCH], f32, tag="ps2")
                        for kt in range(HT):
                            nc.tensor.matmul(
                                ps[:],
                                wh_sb[:, l, kt, mt, :],
                                hin[kt][:],
                                start=(kt == 0),
                                stop=(kt == HT - 1),
                            )
                        hsb = hpool.tile([128, NCH], mdt, tag=f"h{l + 2}_{mt}")
                        nc.scalar.activation(
                            hsb[:],
                            ps[:],
                            Relu,
                            bias=v_sb[:, COL_C(l) + mt : COL_C(l) + mt + 1],
                            scale=v_sb[:, COL_S(l) + mt : COL_S(l) + mt + 1],
                        )
                        hout.append(hsb)
                    hin = hout

                # ---- out: delta^T = W3^T h + b3; out = delta^T + zlast^T ----
                for lt in range(LT):
                    ps = pspool.tile([128, NCH], f32, tag="pso")
                    for kt in range(HT):
                        nc.tensor.matmul(
                            ps[:],
                            w3_sb[:, kt, lt * 128 : (lt + 1) * 128],
                            hin[kt][:],
                            start=(kt == 0),
                            stop=(kt == HT - 1),
                        )
                    ot = opool.tile([128, NCH], f32, tag=f"o{lt}")
                    nc.vector.tensor_tensor(ot[:], ps[:], zts[lt][:], add)
                    nc.vector.tensor_scalar_add(
                        ot[:], ot[:], v_sb[:, COL_B3 + lt : COL_B3 + lt + 1]
                    )
                    nc.sync.dma_start(
                        out=outT[lt * 128 : (lt + 1) * 128, b0 : b0 + NCH], in_=ot[:]
                    )
    nc.compile()
    return nc


def _mdt_np(dt_mode):
    if dt_mode == "bf16":
        import ml_dtypes

        return ml_dtypes.bfloat16
    return np.float32


def prep_core_inputs(
    z_hist, a_hist, W1, b1, Wh, bh, gamma, beta, rmean, rvar, W3, b3, dt_mode=DT_MODE
):
    """Host-side shard prep: returns per-model input dicts (xT/zT shared)."""
    mnp = _mdt_np(dt_mode)
    batch = z_hist.shape[0]
    x = np.concatenate(
        [z_hist.reshape(batch, -1), a_hist.reshape(batch, -1)], axis=1
    ).astype(np.float32)
    xT = np.zeros((DINP, batch), mnp)
    xT[:DIN] = x.T.astype(mnp)
    a_flat = x[:, KTZ * 128 :]  # [batch, 5]
    zT = np.ascontiguousarray(x.T[ZROW0 : ZROW0 + L]).astype(np.float32)

    rstd = 1.0 / np.sqrt(rvar.astype(np.float64) + EPS)  # [NHL, M, HID]
    s_aff = (gamma * rstd).astype(np.float32)
    c_aff = ((bh - rmean) * gamma * rstd + beta).astype(np.float32)

    in_maps = []
    for m in range(M):
        w1p = np.zeros((DINP, HID), np.float32)
        w1p[:DIN] = W1[m]
        w1h = np.ascontiguousarray(
            w1p.reshape(KT1, 128, HID).transpose(1, 0, 2)
        ).astype(mnp)  # [128, KT1, HID]

        whh = np.ascontiguousarray(
            Wh[:, m].reshape(NHL, HT, 128, HT, 128).transpose(2, 0, 1, 3, 4)
        ).astype(mnp)  # [128, NHL, kt, mt, 128]

        w3h = np.ascontiguousarray(W3[m].reshape(HT, 128, L).transpose(1, 0, 2)).astype(
            mnp
        )  # [128, HT, L]

        vecs = np.zeros((128, NVEC), np.float32)
        vecs[:, COL_B1 : COL_B1 + HT] = b1[m].reshape(HT, 128).T
        for l in range(NHL):
            vecs[:, COL_S(l) : COL_S(l) + HT] = s_aff[l, m].reshape(HT, 128).T
            vecs[:, COL_C(l) : COL_C(l) + HT] = c_aff[l, m].reshape(HT, 128).T
        vecs[:, COL_B3 : COL_B3 + LT] = b3[m].reshape(LT, 128).T

        y_a = (a_flat @ W1[m][KTZ * 128 :]).T  # [HID, batch] fp32
        yah = np.ascontiguousarray(
            y_a.reshape(HT, 128, batch).transpose(1, 0, 2)
        ).astype(mnp)  # [128, HT, batch]
        in_maps.append(
            {"xT": xT, "zT": zT, "ya": yah, "w1": w1h, "wh": whh, "w3": w3h,
             "vecs": vecs}
        )
    return in_maps


def _reset_device():
    """Clear any exec-unit wedge a previous (profiled) session left behind."""
    try:
        import ctypes

        import jax

        jax.devices()
        lib = ctypes.CDLL("/opt/axon/libaxon_pjrt.so")
        if hasattr(lib, "axon_reset"):
            lib.axon_reset.restype = ctypes.c_int64
            lib.axon_reset()
    except Exception:
        pass


def kernel(**inputs):
    inputs = {k: np.asarray(v) for k, v in inputs.items()}
    in_maps = prep_core_inputs(**inputs)
    nc = build_bass(B)

    from concourse import bass_utils

    _reset_device()
    res = bass_utils.run_bass_kernel_spmd(nc, in_maps, core_ids=list(range(M)))
    out = np.stack(
        [np.ascontiguousarray(res.results[m]["outT"].T) for m in range(M)]
    )  # [M, B, L]
    return out.astype(np.float32)
